# revision 14
# baseline (speedup 1.0000x reference)
"""CrossStateFusion Trainium2 kernel.

Sharding: data-parallel over batch B=8 -> one NeuronCore per batch element.
Device kernel (per core) computes, for its batch element:
  x = (state_repr + role_emb[roles]) * valid           (token-major, via PE transpose)
  q,k,v projections (PE, stationary = transposed x tiles, role-bias folded in
  as a rank-1 PSUM seed matmul), attention over the K=4 states per residue
  (DVE products + segmented reduces), softmax with host-precomputed additive
  masks, swr-weighted pooling, max/mean features, gating MLP with LN1/LN2
  (bn_stats), exact erf-gelu and exp-based sigmoid.
Host precomputes only O(B*K*L) mask/weight tensors + transposed weights.
"""

import numpy as np

P = 128
K = 4
L = 1024
D = 512
H = 8
HD = 64
DC = D // P          # 4 feature chunks of 128
NL = L // P          # 8 residue tiles of 128
G1C = (3 * D) // P   # 12 chunks for the 1536-wide gate input
SCALE = HD ** -0.5
EPS = 1e-8
LN_EPS = 1e-5
BIGNEG = 30000.0

_CACHE = {}


def _fv(base, off, dims):
    """AP view: keep base's partition dim, replace free dims, shift offset (elements)."""
    import concourse.bass as bass
    return bass.AP(tensor=base.tensor, offset=base.offset + off, ap=[base.ap[0]] + dims)


def _build(ln1_triv, norm_triv):
    import concourse.bass as bass
    import concourse.tile as tile
    from concourse import mybir

    f32 = mybir.dt.float32
    bf16 = mybir.dt.bfloat16
    AX = mybir.AxisListType
    OP = mybir.AluOpType
    AF = mybir.ActivationFunctionType

    # fvec rows (free-varying vectors replicated across partitions)
    R_BG1 = 0
    R_BG2 = 1
    NF = 2
    R_L1G = R_L1B = R_NG = R_NB = None
    if not ln1_triv:
        R_L1G = NF; NF += 3
        R_L1B = NF; NF += 3
    if not norm_triv:
        R_NG = NF; NF += 1
        R_NB = NF; NF += 1

    from concourse import bacc
    nc = bacc.Bacc("TRN2", target_bir_lowering=False)
    dp = nc.declare_dram_parameter
    sr = dp("sr", [K, L, D], f32, isOutput=False)
    aux = dp("aux", [L, 33], f32, isOutput=False)
    rball = dp("rball", [K, 4, D], f32, isOutput=False)
    wqT = dp("wqT", [D, D], f32, isOutput=False)
    wkT = dp("wkT", [D, D], f32, isOutput=False)
    wvT = dp("wvT", [D, D], f32, isOutput=False)
    woT = dp("woT", [D, D], f32, isOutput=False)
    wg1T = dp("wg1T", [3 * D, D], f32, isOutput=False)
    wg2T = dp("wg2T", [D, D], f32, isOutput=False)
    fvec = dp("fvec", [NF, D], f32, isOutput=False)
    eye = dp("eye", [P, P], f32, isOutput=False)
    selk = dp("selk", [K, K, P], f32, isOutput=False)
    fused_o = dp("fused", [L, D], f32, isOutput=True)
    apool_o = dp("apool", [L, D], f32, isOutput=True)
    wmean_o = dp("wmean", [L, D], f32, isOutput=True)
    mfeat_o = dp("mfeat", [L, D], f32, isOutput=True)
    amean_o = dp("amean", [L, K], f32, isOutput=True)

    with tile.TileContext(nc) as tc:
        import contextlib
        with contextlib.ExitStack() as ctx:
            wp = ctx.enter_context(tc.tile_pool(name="wp", bufs=1))
            work = ctx.enter_context(tc.tile_pool(name="work", bufs=2))
            psum = ctx.enter_context(tc.tile_pool(name="psum", bufs=1, space="PSUM"))

            # ---- persistent weights ----
            wq_sb = wp.tile([P, DC, D], f32)
            wk_sb = wp.tile([P, DC, D], f32)
            wv_sb = wp.tile([P, DC, D], f32)
            wo_sb = wp.tile([P, DC, D], f32)
            wg1_sb = wp.tile([P, G1C, D], f32)
            wg2_sb = wp.tile([P, DC, D], f32)
            nc.gpsimd.dma_start(out=wq_sb[:], in_=wqT[:, :].rearrange("(c p) n -> p c n", p=P))
            nc.gpsimd.dma_start(out=wk_sb[:], in_=wkT[:, :].rearrange("(c p) n -> p c n", p=P))
            nc.gpsimd.dma_start(out=wv_sb[:], in_=wvT[:, :].rearrange("(c p) n -> p c n", p=P))
            nc.gpsimd.dma_start(out=wo_sb[:], in_=woT[:, :].rearrange("(c p) n -> p c n", p=P))
            nc.gpsimd.dma_start(out=wg1_sb[:], in_=wg1T[:, :].rearrange("(c p) n -> p c n", p=P))
            nc.gpsimd.dma_start(out=wg2_sb[:], in_=wg2T[:, :].rearrange("(c p) n -> p c n", p=P))
            fvec_sb = wp.tile([P, NF, D], f32)
            fv_dram = fvec[:, :]
            nc.gpsimd.dma_start(
                out=fvec_sb[:],
                in_=bass.AP(tensor=fv_dram.tensor, offset=fv_dram.offset,
                            ap=[[0, P]] + fv_dram.ap),
            )
            eye_sb = wp.tile([P, P], f32)
            nc.gpsimd.dma_start(out=eye_sb[:], in_=eye[:, :])
            # role-bias rows packed [k_partition, proj(q,k,v,tok), D]; seeds are
            # one-hot-selector matmuls broadcasting row k to all 128 partitions
            # stage through DVE so PE's seed matmuls wait on one semaphore only
            rball_ld = wp.tile([K, 4, D], f32)
            nc.gpsimd.dma_start(out=rball_ld[:], in_=rball[:, :, :])
            selk_ld = wp.tile([K, K, P], f32)
            nc.gpsimd.dma_start(out=selk_ld[:], in_=selk[:, :, :])
            rball_sb = wp.tile([K, 4, D], f32)
            nc.vector.tensor_copy(rball_sb[:], rball_ld[:])
            selk_sb = wp.tile([K, K, P], f32)
            nc.vector.tensor_copy(selk_sb[:], selk_ld[:])
            eps_t = wp.tile([P, 1], f32)
            nc.vector.memset(eps_t[:], LN_EPS)

            for t in range(NL):
                ls = t * P
                aux_t = work.tile([P, 33], f32, tag="aux")
                nc.sync.dma_start(out=aux_t[:], in_=aux[ls:ls + P, :])
                srT = []
                for k in range(K):
                    s = work.tile([P, DC, P], f32, tag="srT", bufs=3)
                    for c in range(DC):
                        nc.sync.dma_start(
                            out=s[:, c, :],
                            in_=sr[k, ls:ls + P, c * P:(c + 1) * P].rearrange("l p -> p l"))
                    srT.append(s)

                Q4 = work.tile([P, K, D], bf16, tag="Q4", bufs=1)
                K4 = work.tile([P, K, D], bf16, tag="K4")
                V4 = work.tile([P, K, D], f32, tag="V4")
                X4 = work.tile([P, K, D], f32, tag="X4", bufs=1)

                for k in range(K):
                    pq = psum.tile([P, D], f32, tag="pq", bufs=1)
                    pk = psum.tile([P, D], f32, tag="pk", bufs=1)
                    pv = psum.tile([P, D], f32, tag="pv", bufs=1)
                    px = psum.tile([P, D], f32, tag="px", bufs=1)
                    # rank-1 seeds: PSUM <- selector^T @ rb rows (role bias broadcast)
                    sel = selk_sb[:, k, :]
                    nc.tensor.matmul(pq[:], sel, rball_sb[:, 0, :],
                                     start=True, stop=False)
                    nc.tensor.matmul(pk[:], sel, rball_sb[:, 1, :],
                                     start=True, stop=False)
                    nc.tensor.matmul(pv[:], sel, rball_sb[:, 2, :],
                                     start=True, stop=False)
                    nc.tensor.matmul(px[:], sel, rball_sb[:, 3, :],
                                     start=True, stop=False, skip_group_check=True)
                    for c in range(DC):
                        lhs = srT[k][:, c, :]
                        last = c == DC - 1
                        nc.tensor.matmul(pq[:], lhs, wq_sb[:, c, :], start=False, stop=last)
                        nc.tensor.matmul(pk[:], lhs, wk_sb[:, c, :], start=False, stop=last)
                        nc.tensor.matmul(pv[:], lhs, wv_sb[:, c, :], start=False, stop=last)
                        nc.tensor.matmul(px[:, c * P:(c + 1) * P], lhs, eye_sb[:],
                                         start=False, stop=True, skip_group_check=True)
                    nc.scalar.copy(Q4[:, k, :], pq[:])
                    nc.scalar.copy(K4[:, k, :], pk[:])
                    nc.vector.tensor_copy(V4[:, k, :], pv[:])
                    # x = (sr + rb) * valid  (valid is a per-partition scalar here)
                    nc.scalar.activation(X4[:, k, :], px[:], AF.Copy,
                                         scale=aux_t[:, k:k + 1])

                # ---- attention over the K states of each residue ----
                LT = work.tile([P, K, K, H], f32, tag="LT")
                k4v = K4[:].rearrange("p j (h d) -> p j h d", h=H)
                for i in range(K):
                    Pi = work.tile([P, K, H, HD], bf16, tag="Pi")
                    nc.vector.tensor_mul(
                        Pi[:], _fv(Q4[:], i * D, [[0, K], [HD, H], [1, HD]]), k4v)
                    nc.vector.tensor_reduce(LT[:, i, :, :], Pi[:], axis=AX.X, op=OP.add)
                # logits = qk*scale*valid_i*valid_j + log(nsw_j) - BIG*(1-valid_j)
                nc.vector.tensor_mul(LT[:], LT[:], _fv(aux_t[:], 8, [[4, K], [1, K], [0, H]]))
                nc.vector.tensor_add(LT[:], LT[:], _fv(aux_t[:], 24, [[0, K], [1, K], [0, H]]))
                M_ = work.tile([P, K, H], f32, tag="M")
                nc.vector.tensor_reduce(
                    M_[:], _fv(LT[:], 0, [[K * H, K], [1, H], [H, K]]), axis=AX.X, op=OP.max)
                E_ = work.tile([P, K, K, H], f32, tag="E")
                nc.vector.tensor_sub(E_[:], LT[:], _fv(M_[:], 0, [[H, K], [0, K], [1, H]]))
                A_ = work.tile([P, K, K, H], f32, tag="A")
                nc.scalar.activation(A_[:], E_[:], AF.Exp)
                S_ = work.tile([P, K, H], f32, tag="S")
                nc.vector.tensor_reduce(
                    S_[:], _fv(A_[:], 0, [[K * H, K], [1, H], [H, K]]), axis=AX.X, op=OP.add)
                RS = work.tile([P, K, H], f32, tag="RS")
                nc.vector.reciprocal(RS[:], S_[:])
                nc.vector.tensor_mul(A_[:], A_[:], _fv(RS[:], 0, [[H, K], [0, K], [1, H]]))
                nc.vector.tensor_mul(A_[:], A_[:], _fv(aux_t[:], 0, [[0, K], [1, K], [0, H]]))
                AM = work.tile([P, K], f32, tag="AM")
                nc.vector.tensor_reduce(
                    AM[:], _fv(A_[:], 0, [[H, K], [K * H, K], [1, H]]), axis=AX.XY, op=OP.add)
                nc.vector.tensor_scalar_mul(AM[:], AM[:], 1.0 / (K * H))
                nc.sync.dma_start(out=amean_o[ls:ls + P, :], in_=AM[:])
                AW = work.tile([P, K, K, H], f32, tag="AW")
                nc.vector.tensor_mul(AW[:], A_[:], _fv(aux_t[:], 4, [[1, K], [0, K], [0, H]]))
                W_ = work.tile([P, K, H], f32, tag="W")
                nc.vector.tensor_reduce(
                    W_[:], _fv(AW[:], 0, [[H, K], [1, H], [K * H, K]]), axis=AX.X, op=OP.add)
                Z_ = work.tile([P, D], f32, tag="Z")
                nc.vector.tensor_mul(Z_[:], V4[:, 0, :], _fv(W_[:], 0, [[1, H], [0, HD]]))
                for j in range(1, K):
                    TZ = work.tile([P, D], f32, tag="TZ")
                    nc.vector.tensor_mul(TZ[:], V4[:, j, :], _fv(W_[:], j * H, [[1, H], [0, HD]]))
                    nc.vector.tensor_add(Z_[:], Z_[:], TZ[:])

                # ---- pooled features: G = [attn_pooled | weighted_mean | max_feat] ----
                G = work.tile([P, 3, D], f32, tag="G")
                nc.vector.tensor_scalar_mul(G[:, 1, :], X4[:, 0, :], aux_t[:, 4:5])
                for k in range(1, K):
                    nc.vector.scalar_tensor_tensor(
                        G[:, 1, :], X4[:, k, :], aux_t[:, 4 + k:5 + k], G[:, 1, :],
                        OP.mult, OP.add)
                nc.vector.tensor_scalar_add(G[:, 2, :], X4[:, 0, :], aux_t[:, 28:29])
                for k in range(1, K):
                    nc.vector.scalar_tensor_tensor(
                        G[:, 2, :], X4[:, k, :], aux_t[:, 28 + k:29 + k], G[:, 2, :],
                        OP.add, OP.max)
                nc.vector.tensor_scalar_mul(G[:, 2, :], G[:, 2, :], aux_t[:, 32:33])

                # attn_pooled = z @ Wo^T : transpose z via PE, then contract
                zT = work.tile([P, DC, P], f32, tag="zT")
                for c in range(DC):
                    pt = psum.tile([P, P], f32, tag="pt", bufs=2)
                    nc.tensor.matmul(pt[:], Z_[:, c * P:(c + 1) * P], eye_sb[:],
                                     start=True, stop=True)
                    nc.vector.tensor_copy(zT[:, c, :], pt[:])
                pmm = psum.tile([P, D], f32, tag="pmm", bufs=2)
                for c in range(DC):
                    nc.tensor.matmul(pmm[:], zT[:, c, :], wo_sb[:, c, :],
                                     start=(c == 0), stop=(c == DC - 1))
                nc.scalar.copy(G[:, 0, :], pmm[:])

                # ---- LN1 over 1536 ----
                st1 = work.tile([P, 3, 6], f32, tag="st1")
                for i in range(3):
                    nc.vector.bn_stats(st1[:, i, :], G[:, i, :])
                mv1 = work.tile([P, 2], f32, tag="mv1")
                nc.vector.bn_aggr(mv1[:], st1[:])
                r1 = work.tile([P, 1], f32, tag="r1")
                nc.scalar.activation(r1[:], mv1[:, 1:2], AF.Ln, bias=eps_t[:, 0:1])
                nc.scalar.activation(r1[:], r1[:], AF.Exp, scale=-0.5)
                GN = work.tile([P, 3, D], f32, tag="GN", bufs=1)
                nc.vector.scalar_tensor_tensor(
                    GN[:], G[:], mv1[:, 0:1], r1[:].to_broadcast((P, 3, D)),
                    OP.subtract, OP.mult)
                if not ln1_triv:
                    nc.vector.tensor_mul(GN[:], GN[:], fvec_sb[:, R_L1G:R_L1G + 3, :])
                    nc.vector.tensor_add(GN[:], GN[:], fvec_sb[:, R_L1B:R_L1B + 3, :])

                # ---- h = gelu(GN @ Wg1^T + bg1) ----
                GNf = GN[:].rearrange("p a b -> p (a b)")
                gnT = work.tile([P, G1C, P], f32, tag="gnT", bufs=1)
                for c in range(G1C):
                    pt = psum.tile([P, P], f32, tag="pt", bufs=2)
                    nc.tensor.matmul(pt[:], GNf[:, c * P:(c + 1) * P], eye_sb[:],
                                     start=True, stop=True)
                    nc.vector.tensor_copy(gnT[:, c, :], pt[:])
                ph = psum.tile([P, D], f32, tag="pmm", bufs=2)
                for c in range(G1C):
                    nc.tensor.matmul(ph[:], gnT[:, c, :], wg1_sb[:, c, :],
                                     start=(c == 0), stop=(c == G1C - 1))
                HB = work.tile([P, D], f32, tag="mlp512", bufs=6)
                nc.vector.tensor_add(HB[:], ph[:], fvec_sb[:, R_BG1, :])
                EG = work.tile([P, D], f32, tag="mlp512", bufs=6)
                nc.scalar.activation(EG[:], HB[:], AF.Erf, scale=0.7071067811865476)
                # (1+erf)*hb == 2*gelu ; the 0.5 is folded into wg2T host-side
                Hh = work.tile([P, D], f32, tag="mlp512", bufs=6)
                nc.vector.scalar_tensor_tensor(Hh[:], EG[:], 1.0, HB[:], OP.add, OP.mult)

                # ---- gate = sigmoid(h @ Wg2^T + bg2) ----
                hT = work.tile([P, DC, P], f32, tag="hT")
                for c in range(DC):
                    pt = psum.tile([P, P], f32, tag="pt", bufs=2)
                    nc.tensor.matmul(pt[:], Hh[:, c * P:(c + 1) * P], eye_sb[:],
                                     start=True, stop=True)
                    nc.vector.tensor_copy(hT[:, c, :], pt[:])
                pg = psum.tile([P, D], f32, tag="pmm", bufs=2)
                for c in range(DC):
                    nc.tensor.matmul(pg[:], hT[:, c, :], wg2_sb[:, c, :],
                                     start=(c == 0), stop=(c == DC - 1))
                GB = work.tile([P, D], f32, tag="mlp512", bufs=6)
                nc.vector.tensor_add(GB[:], pg[:], fvec_sb[:, R_BG2, :])
                SG = work.tile([P, D], f32, tag="mlp512", bufs=6)
                nc.scalar.activation(SG[:], GB[:], AF.Exp, scale=-1.0)
                nc.vector.tensor_scalar_add(SG[:], SG[:], 1.0)
                GATE = work.tile([P, D], f32, tag="mlp512", bufs=6)
                nc.vector.reciprocal(GATE[:], SG[:])

                # ---- fused = LN2(gate*(0.5*(ap+wm)) + (1-gate)*mf + wm) ----
                S1 = work.tile([P, D], f32, tag="mlp512", bufs=6)
                nc.vector.tensor_add(S1[:], G[:, 0, :], G[:, 1, :])
                U_ = work.tile([P, D], f32, tag="mlp512", bufs=6)
                nc.vector.scalar_tensor_tensor(U_[:], S1[:], 0.5, G[:, 2, :],
                                               OP.mult, OP.subtract)
                FI = work.tile([P, D], f32, tag="FI")
                nc.vector.tensor_mul(FI[:], GATE[:], U_[:])
                nc.vector.tensor_add(FI[:], FI[:], G[:, 2, :])
                nc.vector.tensor_add(FI[:], FI[:], G[:, 1, :])
                st2 = work.tile([P, 6], f32, tag="st2")
                nc.vector.bn_stats(st2[:], FI[:])
                mv2 = work.tile([P, 2], f32, tag="mv2")
                nc.vector.bn_aggr(mv2[:], st2[:])
                r2 = work.tile([P, 1], f32, tag="r2")
                nc.scalar.activation(r2[:], mv2[:, 1:2], AF.Ln, bias=eps_t[:, 0:1])
                nc.scalar.activation(r2[:], r2[:], AF.Exp, scale=-0.5)
                FN = work.tile([P, D], f32, tag="FN")
                nc.vector.scalar_tensor_tensor(
                    FN[:], FI[:], mv2[:, 0:1], r2[:].to_broadcast((P, D)),
                    OP.subtract, OP.mult)
                if not norm_triv:
                    nc.vector.tensor_mul(FN[:], FN[:], fvec_sb[:, R_NG, :])
                    nc.vector.tensor_add(FN[:], FN[:], fvec_sb[:, R_NB, :])

                nc.sync.dma_start(out=fused_o[ls:ls + P, :], in_=FN[:])
                nc.sync.dma_start(out=apool_o[ls:ls + P, :], in_=G[:, 0, :])
                nc.sync.dma_start(out=wmean_o[ls:ls + P, :], in_=G[:, 1, :])
                nc.sync.dma_start(out=mfeat_o[ls:ls + P, :], in_=G[:, 2, :])

    nc.compile()
    return nc, (NF, R_BG1, R_BG2, R_L1G, R_L1B, R_NG, R_NB)


def kernel(**inputs):
    from concourse.bass_utils import run_bass_kernel_spmd

    sr = np.ascontiguousarray(np.asarray(inputs["state_repr"], dtype=np.float32))
    rmask = np.asarray(inputs["residue_mask"])
    sw = np.asarray(inputs["state_weights"], dtype=np.float32)
    roles = np.asarray(inputs["state_roles"])
    pres = np.asarray(inputs["state_present_mask"])
    remb = np.asarray(inputs["role_emb"], dtype=np.float32)
    Wq = np.asarray(inputs["Wq"], dtype=np.float32)
    Wk = np.asarray(inputs["Wk"], dtype=np.float32)
    Wv = np.asarray(inputs["Wv"], dtype=np.float32)
    Wo = np.asarray(inputs["Wo"], dtype=np.float32)
    ln1_g = np.asarray(inputs["ln1_g"], dtype=np.float32)
    ln1_b = np.asarray(inputs["ln1_b"], dtype=np.float32)
    Wg1 = np.asarray(inputs["Wg1"], dtype=np.float32)
    bg1 = np.asarray(inputs["bg1"], dtype=np.float32)
    Wg2 = np.asarray(inputs["Wg2"], dtype=np.float32)
    bg2 = np.asarray(inputs["bg2"], dtype=np.float32)
    norm_g = np.asarray(inputs["norm_g"], dtype=np.float32)
    norm_b = np.asarray(inputs["norm_b"], dtype=np.float32)

    B = sr.shape[0]

    # ---- host-side small math (matches reference) ----
    presf = pres.astype(np.float32)
    w = sw * presf
    denom = w.sum(1, keepdims=True)
    fallback = presf / np.maximum(presf.sum(1, keepdims=True), 1.0)
    nsw = np.where(denom > EPS, w / np.maximum(denom, EPS), fallback).astype(np.float32)

    valid = (rmask & pres[:, :, None]).astype(np.float32)          # [B,K,L]
    vlk = np.ascontiguousarray(valid.transpose(0, 2, 1))           # [B,L,K]
    swr = nsw[:, None, :] * vlk
    swr = swr / np.maximum(swr.sum(-1, keepdims=True), EPS)
    logw = np.log(np.maximum(nsw, EPS))
    cb = logw[:, None, :] + (vlk - 1.0) * BIGNEG
    vvs = (SCALE * vlk[:, :, :, None] * vlk[:, :, None, :]).reshape(B, L, K * K)
    madd = (vlk - 1.0) * 1e9
    hasany = vlk.max(-1, keepdims=True)
    aux = np.ascontiguousarray(
        np.concatenate([vlk, swr, vvs, cb, madd, hasany], axis=-1).astype(np.float32))

    role_e = remb[np.maximum(roles, 0)].astype(np.float32)         # [B,K,D]
    rbq = np.einsum("bkd,od->bko", role_e, Wq).astype(np.float32)
    rbk = np.einsum("bkd,od->bko", role_e, Wk).astype(np.float32)
    rbv = np.einsum("bkd,od->bko", role_e, Wv).astype(np.float32)

    ln1_triv = bool(np.allclose(ln1_g, 1.0) and np.allclose(ln1_b, 0.0))
    norm_triv = bool(np.allclose(norm_g, 1.0) and np.allclose(norm_b, 0.0))

    key = (ln1_triv, norm_triv)
    if key not in _CACHE:
        _CACHE[key] = _build(*key)
    nc, (NF, R_BG1, R_BG2, R_L1G, R_L1B, R_NG, R_NB) = _CACHE[key]

    fvec = np.zeros((NF, D), np.float32)
    fvec[R_BG1] = bg1
    fvec[R_BG2] = bg2
    if not ln1_triv:
        fvec[R_L1G:R_L1G + 3] = ln1_g.reshape(3, D)
        fvec[R_L1B:R_L1B + 3] = ln1_b.reshape(3, D)
    if not norm_triv:
        fvec[R_NG] = norm_g
        fvec[R_NB] = norm_b

    common = {
        "wqT": np.ascontiguousarray(Wq.T),
        "wkT": np.ascontiguousarray(Wk.T),
        "wvT": np.ascontiguousarray(Wv.T),
        "woT": np.ascontiguousarray(Wo.T),
        "wg1T": np.ascontiguousarray(Wg1.T),
        "wg2T": np.ascontiguousarray(0.5 * Wg2.T),
        "fvec": fvec,
        "eye": np.eye(P, dtype=np.float32),
        "selk": np.ascontiguousarray(
            np.eye(K, dtype=np.float32)[:, :, None] * np.ones((1, 1, P), np.float32)),
    }
    in_maps = []
    for b in range(B):
        m = dict(common)
        m["sr"] = np.ascontiguousarray(sr[b])
        m["aux"] = np.ascontiguousarray(aux[b])
        m["rball"] = np.ascontiguousarray(
            np.stack([rbq[b], rbk[b], rbv[b], role_e[b]], axis=1))
        in_maps.append(m)

    res = run_bass_kernel_spmd(nc, in_maps, core_ids=list(range(B)))
    r = res.results
    fused = np.stack([r[b]["fused"] for b in range(B)])
    apool = np.stack([r[b]["apool"] for b in range(B)])
    wmean = np.stack([r[b]["wmean"] for b in range(B)])
    mfeat = np.stack([r[b]["mfeat"] for b in range(B)])
    amean = np.stack([r[b]["amean"] for b in range(B)])
    return fused, apool, wmean, mfeat, amean, nsw


# revision 18
# speedup vs baseline: 73.4111x; 73.4111x over previous
"""CrossStateFusion Trainium2 kernel.

Sharding: data-parallel over batch B=8 -> one NeuronCore per batch element.
Device kernel (per core) computes, for its batch element:
  x = (state_repr + role_emb[roles]) * valid           (token-major, via PE transpose)
  q,k,v projections (PE, stationary = transposed x tiles, role-bias folded in
  as a rank-1 PSUM seed matmul), attention over the K=4 states per residue
  (DVE products + segmented reduces), softmax with host-precomputed additive
  masks, swr-weighted pooling, max/mean features, gating MLP with LN1/LN2
  (bn_stats), exact erf-gelu and exp-based sigmoid.
Host precomputes only O(B*K*L) mask/weight tensors + transposed weights.
"""

import numpy as np

P = 128
K = 4
L = 1024
D = 512
H = 8
HD = 64
DC = D // P          # 4 feature chunks of 128
NL = L // P          # 8 residue tiles of 128
G1C = (3 * D) // P   # 12 chunks for the 1536-wide gate input
SCALE = HD ** -0.5
EPS = 1e-8
LN_EPS = 1e-5
BIGNEG = 30000.0

_CACHE = {}
LAST_RESULTS = None


def _fv(base, off, dims):
    """AP view: keep base's partition dim, replace free dims, shift offset (elements)."""
    import concourse.bass as bass
    return bass.AP(tensor=base.tensor, offset=base.offset + off, ap=[base.ap[0]] + dims)


def _build(ln1_triv, norm_triv):
    import concourse.bass as bass
    import concourse.tile as tile
    from concourse import mybir

    f32 = mybir.dt.float32
    bf16 = mybir.dt.bfloat16
    AX = mybir.AxisListType
    OP = mybir.AluOpType
    AF = mybir.ActivationFunctionType

    # fvec rows (free-varying vectors replicated across partitions)
    R_BG1 = 0
    R_BG2 = 1
    NF = 2
    R_L1G = R_L1B = R_NG = R_NB = None
    if not ln1_triv:
        R_L1G = NF; NF += 3
        R_L1B = NF; NF += 3
    if not norm_triv:
        R_NG = NF; NF += 1
        R_NB = NF; NF += 1

    from concourse import bacc
    nc = bacc.Bacc("TRN2", target_bir_lowering=False)
    dp = nc.declare_dram_parameter
    sr = dp("sr", [K, L, D], f32, isOutput=False)
    aux = dp("aux", [L, 33], f32, isOutput=False)
    rball = dp("rball", [K, 4, D], f32, isOutput=False)
    wqT = dp("wqT", [D, D], f32, isOutput=False)
    wkT = dp("wkT", [D, D], f32, isOutput=False)
    wvT = dp("wvT", [D, D], f32, isOutput=False)
    woT = dp("woT", [D, D], f32, isOutput=False)
    wg1T = dp("wg1T", [3 * D, D], f32, isOutput=False)
    wg2T = dp("wg2T", [D, D], f32, isOutput=False)
    fvec = dp("fvec", [NF, D], f32, isOutput=False)
    eye = dp("eye", [P, P], f32, isOutput=False)
    selk = dp("selk", [K, K, P], f32, isOutput=False)
    fused_o = dp("fused", [L, D], f32, isOutput=True)
    apool_o = dp("apool", [L, D], f32, isOutput=True)
    wmean_o = dp("wmean", [L, D], f32, isOutput=True)
    mfeat_o = dp("mfeat", [L, D], f32, isOutput=True)
    amean_o = dp("amean", [L, K], f32, isOutput=True)

    with tile.TileContext(nc) as tc:
        import contextlib
        with contextlib.ExitStack() as ctx:
            wp = ctx.enter_context(tc.tile_pool(name="wp", bufs=1))
            work = ctx.enter_context(tc.tile_pool(name="work", bufs=2))
            psum = ctx.enter_context(tc.tile_pool(name="psum", bufs=1, space="PSUM"))

            # ---- persistent weights ----
            wq_sb = wp.tile([P, DC, D], f32)
            wk_sb = wp.tile([P, DC, D], f32)
            wv_sb = wp.tile([P, DC, D], f32)
            wo_sb = wp.tile([P, DC, D], f32)
            wg1_sb = wp.tile([P, G1C, D], f32)
            wg2_sb = wp.tile([P, DC, D], f32)
            nc.gpsimd.dma_start(out=wq_sb[:], in_=wqT[:, :].rearrange("(c p) n -> p c n", p=P))
            nc.gpsimd.dma_start(out=wk_sb[:], in_=wkT[:, :].rearrange("(c p) n -> p c n", p=P))
            nc.gpsimd.dma_start(out=wv_sb[:], in_=wvT[:, :].rearrange("(c p) n -> p c n", p=P))
            nc.gpsimd.dma_start(out=wo_sb[:], in_=woT[:, :].rearrange("(c p) n -> p c n", p=P))
            nc.gpsimd.dma_start(out=wg1_sb[:], in_=wg1T[:, :].rearrange("(c p) n -> p c n", p=P))
            nc.gpsimd.dma_start(out=wg2_sb[:], in_=wg2T[:, :].rearrange("(c p) n -> p c n", p=P))
            fvec_sb = wp.tile([P, NF, D], f32)
            fv_dram = fvec[:, :]
            nc.gpsimd.dma_start(
                out=fvec_sb[:],
                in_=bass.AP(tensor=fv_dram.tensor, offset=fv_dram.offset,
                            ap=[[0, P]] + fv_dram.ap),
            )
            eye_sb = wp.tile([P, P], f32)
            nc.gpsimd.dma_start(out=eye_sb[:], in_=eye[:, :])
            # role-bias rows packed [k_partition, proj(q,k,v,tok), D]; seeds are
            # one-hot-selector matmuls broadcasting row k to all 128 partitions
            # stage through DVE so PE's seed matmuls wait on one semaphore only
            rball_ld = wp.tile([K, 4, D], f32)
            nc.gpsimd.dma_start(out=rball_ld[:], in_=rball[:, :, :])
            selk_ld = wp.tile([K, K, P], f32)
            nc.gpsimd.dma_start(out=selk_ld[:], in_=selk[:, :, :])
            rball_sb = wp.tile([K, 4, D], f32)
            nc.vector.tensor_copy(rball_sb[:], rball_ld[:])
            selk_sb = wp.tile([K, K, P], f32)
            nc.vector.tensor_copy(selk_sb[:], selk_ld[:])
            eps_t = wp.tile([P, 1], f32)
            nc.vector.memset(eps_t[:], LN_EPS)

            for t in range(NL):
                ls = t * P
                aux_t = work.tile([P, 33], f32, tag="aux")
                nc.sync.dma_start(out=aux_t[:], in_=aux[ls:ls + P, :])
                srT = []
                for k in range(K):
                    s = work.tile([P, DC, P], f32, tag="srT", bufs=3)
                    for c in range(DC):
                        nc.sync.dma_start(
                            out=s[:, c, :],
                            in_=sr[k, ls:ls + P, c * P:(c + 1) * P].rearrange("l p -> p l"))
                    srT.append(s)

                Q4 = work.tile([P, K, D], bf16, tag="Q4", bufs=1)
                K4 = work.tile([P, K, D], bf16, tag="K4")
                V4 = work.tile([P, K, D], f32, tag="V4")
                X4 = work.tile([P, K, D], f32, tag="X4", bufs=1)

                for k in range(K):
                    pq = psum.tile([P, D], f32, tag="pq", bufs=1)
                    pk = psum.tile([P, D], f32, tag="pk", bufs=1)
                    pv = psum.tile([P, D], f32, tag="pv", bufs=1)
                    px = psum.tile([P, D], f32, tag="px", bufs=1)
                    # rank-1 seeds: PSUM <- selector^T @ rb rows (role bias broadcast)
                    sel = selk_sb[:, k, :]
                    nc.tensor.matmul(pq[:], sel, rball_sb[:, 0, :],
                                     start=True, stop=False)
                    nc.tensor.matmul(pk[:], sel, rball_sb[:, 1, :],
                                     start=True, stop=False)
                    nc.tensor.matmul(pv[:], sel, rball_sb[:, 2, :],
                                     start=True, stop=False)
                    nc.tensor.matmul(px[:], sel, rball_sb[:, 3, :],
                                     start=True, stop=False, skip_group_check=True)
                    for c in range(DC):
                        lhs = srT[k][:, c, :]
                        last = c == DC - 1
                        nc.tensor.matmul(pq[:], lhs, wq_sb[:, c, :], start=False, stop=last)
                        nc.tensor.matmul(pk[:], lhs, wk_sb[:, c, :], start=False, stop=last)
                        nc.tensor.matmul(pv[:], lhs, wv_sb[:, c, :], start=False, stop=last)
                        nc.tensor.matmul(px[:, c * P:(c + 1) * P], lhs, eye_sb[:],
                                         start=False, stop=True, skip_group_check=True)
                    nc.scalar.copy(Q4[:, k, :], pq[:])
                    nc.scalar.copy(K4[:, k, :], pk[:])
                    nc.vector.tensor_copy(V4[:, k, :], pv[:])
                    # x = (sr + rb) * valid  (valid is a per-partition scalar here)
                    nc.scalar.activation(X4[:, k, :], px[:], AF.Copy,
                                         scale=aux_t[:, k:k + 1])

                # ---- attention over the K states of each residue ----
                LT = work.tile([P, K, K, H], f32, tag="LT")
                k4v = K4[:].rearrange("p j (h d) -> p j h d", h=H)
                for i in range(K):
                    Pi = work.tile([P, K, H, HD], bf16, tag="Pi")
                    nc.vector.tensor_mul(
                        Pi[:], _fv(Q4[:], i * D, [[0, K], [HD, H], [1, HD]]), k4v)
                    nc.vector.tensor_reduce(LT[:, i, :, :], Pi[:], axis=AX.X, op=OP.add)
                # logits = qk*scale*valid_i*valid_j + log(nsw_j) - BIG*(1-valid_j)
                nc.vector.tensor_mul(LT[:], LT[:], _fv(aux_t[:], 8, [[4, K], [1, K], [0, H]]))
                nc.vector.tensor_add(LT[:], LT[:], _fv(aux_t[:], 24, [[0, K], [1, K], [0, H]]))
                M_ = work.tile([P, K, H], f32, tag="M")
                nc.vector.tensor_reduce(
                    M_[:], _fv(LT[:], 0, [[K * H, K], [1, H], [H, K]]), axis=AX.X, op=OP.max)
                E_ = work.tile([P, K, K, H], f32, tag="E")
                nc.vector.tensor_sub(E_[:], LT[:], _fv(M_[:], 0, [[H, K], [0, K], [1, H]]))
                A_ = work.tile([P, K, K, H], f32, tag="A")
                nc.scalar.activation(A_[:], E_[:], AF.Exp)
                S_ = work.tile([P, K, H], f32, tag="S")
                nc.vector.tensor_reduce(
                    S_[:], _fv(A_[:], 0, [[K * H, K], [1, H], [H, K]]), axis=AX.X, op=OP.add)
                RS = work.tile([P, K, H], f32, tag="RS")
                nc.vector.reciprocal(RS[:], S_[:])
                nc.vector.tensor_mul(A_[:], A_[:], _fv(RS[:], 0, [[H, K], [0, K], [1, H]]))
                nc.vector.tensor_mul(A_[:], A_[:], _fv(aux_t[:], 0, [[0, K], [1, K], [0, H]]))
                AM = work.tile([P, K], f32, tag="AM")
                nc.vector.tensor_reduce(
                    AM[:], _fv(A_[:], 0, [[H, K], [K * H, K], [1, H]]), axis=AX.XY, op=OP.add)
                nc.vector.tensor_scalar_mul(AM[:], AM[:], 1.0 / (K * H))
                nc.sync.dma_start(out=amean_o[ls:ls + P, :], in_=AM[:])
                AW = work.tile([P, K, K, H], f32, tag="AW")
                nc.vector.tensor_mul(AW[:], A_[:], _fv(aux_t[:], 4, [[1, K], [0, K], [0, H]]))
                W_ = work.tile([P, K, H], f32, tag="W")
                nc.vector.tensor_reduce(
                    W_[:], _fv(AW[:], 0, [[H, K], [1, H], [K * H, K]]), axis=AX.X, op=OP.add)
                Z_ = work.tile([P, D], f32, tag="Z")
                nc.vector.tensor_mul(Z_[:], V4[:, 0, :], _fv(W_[:], 0, [[1, H], [0, HD]]))
                for j in range(1, K):
                    TZ = work.tile([P, D], f32, tag="TZ")
                    nc.vector.tensor_mul(TZ[:], V4[:, j, :], _fv(W_[:], j * H, [[1, H], [0, HD]]))
                    nc.vector.tensor_add(Z_[:], Z_[:], TZ[:])

                # ---- pooled features: G = [attn_pooled | weighted_mean | max_feat] ----
                G = work.tile([P, 3, D], f32, tag="G")
                nc.vector.tensor_scalar_mul(G[:, 1, :], X4[:, 0, :], aux_t[:, 4:5])
                for k in range(1, K):
                    nc.vector.scalar_tensor_tensor(
                        G[:, 1, :], X4[:, k, :], aux_t[:, 4 + k:5 + k], G[:, 1, :],
                        OP.mult, OP.add)
                nc.vector.tensor_scalar_add(G[:, 2, :], X4[:, 0, :], aux_t[:, 28:29])
                for k in range(1, K):
                    nc.vector.scalar_tensor_tensor(
                        G[:, 2, :], X4[:, k, :], aux_t[:, 28 + k:29 + k], G[:, 2, :],
                        OP.add, OP.max)
                nc.vector.tensor_scalar_mul(G[:, 2, :], G[:, 2, :], aux_t[:, 32:33])

                # attn_pooled = z @ Wo^T : transpose z via PE, then contract
                zT = work.tile([P, DC, P], f32, tag="zT")
                for c in range(DC):
                    pt = psum.tile([P, P], f32, tag="pt", bufs=2)
                    nc.tensor.matmul(pt[:], Z_[:, c * P:(c + 1) * P], eye_sb[:],
                                     start=True, stop=True)
                    nc.vector.tensor_copy(zT[:, c, :], pt[:])
                pmm = psum.tile([P, D], f32, tag="pmm", bufs=2)
                for c in range(DC):
                    nc.tensor.matmul(pmm[:], zT[:, c, :], wo_sb[:, c, :],
                                     start=(c == 0), stop=(c == DC - 1))
                nc.scalar.copy(G[:, 0, :], pmm[:])

                # ---- LN1 over 1536 ----
                st1 = work.tile([P, 3, 6], f32, tag="st1")
                for i in range(3):
                    nc.vector.bn_stats(st1[:, i, :], G[:, i, :])
                mv1 = work.tile([P, 2], f32, tag="mv1")
                nc.vector.bn_aggr(mv1[:], st1[:])
                r1 = work.tile([P, 1], f32, tag="r1")
                nc.scalar.activation(r1[:], mv1[:, 1:2], AF.Ln, bias=eps_t[:, 0:1])
                nc.scalar.activation(r1[:], r1[:], AF.Exp, scale=-0.5)
                GN = work.tile([P, 3, D], f32, tag="GN", bufs=1)
                nc.vector.scalar_tensor_tensor(
                    GN[:], G[:], mv1[:, 0:1], r1[:].to_broadcast((P, 3, D)),
                    OP.subtract, OP.mult)
                if not ln1_triv:
                    nc.vector.tensor_mul(GN[:], GN[:], fvec_sb[:, R_L1G:R_L1G + 3, :])
                    nc.vector.tensor_add(GN[:], GN[:], fvec_sb[:, R_L1B:R_L1B + 3, :])

                # ---- h = gelu(GN @ Wg1^T + bg1) ----
                GNf = GN[:].rearrange("p a b -> p (a b)")
                gnT = work.tile([P, G1C, P], f32, tag="gnT", bufs=1)
                for c in range(G1C):
                    pt = psum.tile([P, P], f32, tag="pt", bufs=2)
                    nc.tensor.matmul(pt[:], GNf[:, c * P:(c + 1) * P], eye_sb[:],
                                     start=True, stop=True)
                    nc.vector.tensor_copy(gnT[:, c, :], pt[:])
                ph = psum.tile([P, D], f32, tag="pmm", bufs=2)
                for c in range(G1C):
                    nc.tensor.matmul(ph[:], gnT[:, c, :], wg1_sb[:, c, :],
                                     start=(c == 0), stop=(c == G1C - 1))
                HB = work.tile([P, D], f32, tag="mlp512", bufs=6)
                nc.vector.tensor_add(HB[:], ph[:], fvec_sb[:, R_BG1, :])
                EG = work.tile([P, D], f32, tag="mlp512", bufs=6)
                nc.scalar.activation(EG[:], HB[:], AF.Erf, scale=0.7071067811865476)
                # (1+erf)*hb == 2*gelu ; the 0.5 is folded into wg2T host-side
                Hh = work.tile([P, D], f32, tag="mlp512", bufs=6)
                nc.vector.scalar_tensor_tensor(Hh[:], EG[:], 1.0, HB[:], OP.add, OP.mult)

                # ---- gate = sigmoid(h @ Wg2^T + bg2) ----
                hT = work.tile([P, DC, P], f32, tag="hT")
                for c in range(DC):
                    pt = psum.tile([P, P], f32, tag="pt", bufs=2)
                    nc.tensor.matmul(pt[:], Hh[:, c * P:(c + 1) * P], eye_sb[:],
                                     start=True, stop=True)
                    nc.vector.tensor_copy(hT[:, c, :], pt[:])
                pg = psum.tile([P, D], f32, tag="pmm", bufs=2)
                for c in range(DC):
                    nc.tensor.matmul(pg[:], hT[:, c, :], wg2_sb[:, c, :],
                                     start=(c == 0), stop=(c == DC - 1))
                GB = work.tile([P, D], f32, tag="mlp512", bufs=6)
                nc.vector.tensor_add(GB[:], pg[:], fvec_sb[:, R_BG2, :])
                SG = work.tile([P, D], f32, tag="mlp512", bufs=6)
                nc.scalar.activation(SG[:], GB[:], AF.Exp, scale=-1.0)
                nc.vector.tensor_scalar_add(SG[:], SG[:], 1.0)
                GATE = work.tile([P, D], f32, tag="mlp512", bufs=6)
                nc.vector.reciprocal(GATE[:], SG[:])

                # ---- fused = LN2(gate*(0.5*(ap+wm)) + (1-gate)*mf + wm) ----
                S1 = work.tile([P, D], f32, tag="mlp512", bufs=6)
                nc.vector.tensor_add(S1[:], G[:, 0, :], G[:, 1, :])
                U_ = work.tile([P, D], f32, tag="mlp512", bufs=6)
                nc.vector.scalar_tensor_tensor(U_[:], S1[:], 0.5, G[:, 2, :],
                                               OP.mult, OP.subtract)
                FI = work.tile([P, D], f32, tag="FI")
                nc.vector.tensor_mul(FI[:], GATE[:], U_[:])
                nc.vector.tensor_add(FI[:], FI[:], G[:, 2, :])
                nc.vector.tensor_add(FI[:], FI[:], G[:, 1, :])
                st2 = work.tile([P, 6], f32, tag="st2")
                nc.vector.bn_stats(st2[:], FI[:])
                mv2 = work.tile([P, 2], f32, tag="mv2")
                nc.vector.bn_aggr(mv2[:], st2[:])
                r2 = work.tile([P, 1], f32, tag="r2")
                nc.scalar.activation(r2[:], mv2[:, 1:2], AF.Ln, bias=eps_t[:, 0:1])
                nc.scalar.activation(r2[:], r2[:], AF.Exp, scale=-0.5)
                FN = work.tile([P, D], f32, tag="FN")
                nc.vector.scalar_tensor_tensor(
                    FN[:], FI[:], mv2[:, 0:1], r2[:].to_broadcast((P, D)),
                    OP.subtract, OP.mult)
                if not norm_triv:
                    nc.vector.tensor_mul(FN[:], FN[:], fvec_sb[:, R_NG, :])
                    nc.vector.tensor_add(FN[:], FN[:], fvec_sb[:, R_NB, :])

                nc.sync.dma_start(out=fused_o[ls:ls + P, :], in_=FN[:])
                nc.sync.dma_start(out=apool_o[ls:ls + P, :], in_=G[:, 0, :])
                nc.sync.dma_start(out=wmean_o[ls:ls + P, :], in_=G[:, 1, :])
                nc.sync.dma_start(out=mfeat_o[ls:ls + P, :], in_=G[:, 2, :])

    nc.compile()
    return nc, (NF, R_BG1, R_BG2, R_L1G, R_L1B, R_NG, R_NB)


def kernel(**inputs):
    sr = np.ascontiguousarray(np.asarray(inputs["state_repr"], dtype=np.float32))
    rmask = np.asarray(inputs["residue_mask"])
    sw = np.asarray(inputs["state_weights"], dtype=np.float32)
    roles = np.asarray(inputs["state_roles"])
    pres = np.asarray(inputs["state_present_mask"])
    remb = np.asarray(inputs["role_emb"], dtype=np.float32)
    Wq = np.asarray(inputs["Wq"], dtype=np.float32)
    Wk = np.asarray(inputs["Wk"], dtype=np.float32)
    Wv = np.asarray(inputs["Wv"], dtype=np.float32)
    Wo = np.asarray(inputs["Wo"], dtype=np.float32)
    ln1_g = np.asarray(inputs["ln1_g"], dtype=np.float32)
    ln1_b = np.asarray(inputs["ln1_b"], dtype=np.float32)
    Wg1 = np.asarray(inputs["Wg1"], dtype=np.float32)
    bg1 = np.asarray(inputs["bg1"], dtype=np.float32)
    Wg2 = np.asarray(inputs["Wg2"], dtype=np.float32)
    bg2 = np.asarray(inputs["bg2"], dtype=np.float32)
    norm_g = np.asarray(inputs["norm_g"], dtype=np.float32)
    norm_b = np.asarray(inputs["norm_b"], dtype=np.float32)

    B = sr.shape[0]

    # ---- host-side small math (matches reference) ----
    presf = pres.astype(np.float32)
    w = sw * presf
    denom = w.sum(1, keepdims=True)
    fallback = presf / np.maximum(presf.sum(1, keepdims=True), 1.0)
    nsw = np.where(denom > EPS, w / np.maximum(denom, EPS), fallback).astype(np.float32)

    valid = (rmask & pres[:, :, None]).astype(np.float32)          # [B,K,L]
    vlk = np.ascontiguousarray(valid.transpose(0, 2, 1))           # [B,L,K]
    swr = nsw[:, None, :] * vlk
    swr = swr / np.maximum(swr.sum(-1, keepdims=True), EPS)
    logw = np.log(np.maximum(nsw, EPS))
    cb = logw[:, None, :] + (vlk - 1.0) * BIGNEG
    vvs = (SCALE * vlk[:, :, :, None] * vlk[:, :, None, :]).reshape(B, L, K * K)
    madd = (vlk - 1.0) * 1e9
    hasany = vlk.max(-1, keepdims=True)
    aux = np.ascontiguousarray(
        np.concatenate([vlk, swr, vvs, cb, madd, hasany], axis=-1).astype(np.float32))

    role_e = remb[np.maximum(roles, 0)].astype(np.float32)         # [B,K,D]
    rbq = np.einsum("bkd,od->bko", role_e, Wq).astype(np.float32)
    rbk = np.einsum("bkd,od->bko", role_e, Wk).astype(np.float32)
    rbv = np.einsum("bkd,od->bko", role_e, Wv).astype(np.float32)

    ln1_triv = bool(np.allclose(ln1_g, 1.0) and np.allclose(ln1_b, 0.0))
    norm_triv = bool(np.allclose(norm_g, 1.0) and np.allclose(norm_b, 0.0))

    key = (ln1_triv, norm_triv)
    if key not in _CACHE:
        _CACHE[key] = _build(*key)
    nc, (NF, R_BG1, R_BG2, R_L1G, R_L1B, R_NG, R_NB) = _CACHE[key]

    fvec = np.zeros((NF, D), np.float32)
    fvec[R_BG1] = bg1
    fvec[R_BG2] = bg2
    if not ln1_triv:
        fvec[R_L1G:R_L1G + 3] = ln1_g.reshape(3, D)
        fvec[R_L1B:R_L1B + 3] = ln1_b.reshape(3, D)
    if not norm_triv:
        fvec[R_NG] = norm_g
        fvec[R_NB] = norm_b

    common = {
        "wqT": np.ascontiguousarray(Wq.T),
        "wkT": np.ascontiguousarray(Wk.T),
        "wvT": np.ascontiguousarray(Wv.T),
        "woT": np.ascontiguousarray(Wo.T),
        "wg1T": np.ascontiguousarray(Wg1.T),
        "wg2T": np.ascontiguousarray(0.5 * Wg2.T),
        "fvec": fvec,
        "eye": np.eye(P, dtype=np.float32),
        "selk": np.ascontiguousarray(
            np.eye(K, dtype=np.float32)[:, :, None] * np.ones((1, 1, P), np.float32)),
    }
    in_maps = []
    for b in range(B):
        m = dict(common)
        m["sr"] = np.ascontiguousarray(sr[b])
        m["aux"] = np.ascontiguousarray(aux[b])
        m["rball"] = np.ascontiguousarray(
            np.stack([rbq[b], rbk[b], rbv[b], role_e[b]], axis=1))
        in_maps.append(m)

    global LAST_IN_MAPS
    LAST_IN_MAPS = in_maps
    r = _run(nc, in_maps, B)
    fused = np.stack([r[b]["fused"] for b in range(B)])
    apool = np.stack([r[b]["apool"] for b in range(B)])
    wmean = np.stack([r[b]["wmean"] for b in range(B)])
    mfeat = np.stack([r[b]["mfeat"] for b in range(B)])
    amean = np.stack([r[b]["amean"] for b in range(B)])
    return fused, apool, wmean, mfeat, amean, nsw


_RUNNER = None
_BENCH_ARGS = None
LAST_IN_MAPS = None


def _get_runner(nc, n_cores):
    """Cached jitted shard_map executor over the 8 NeuronCores (one compile
    per process; repeat kernel() calls only pay transfer + execute)."""
    global _RUNNER
    if _RUNNER is not None:
        return _RUNNER
    import jax
    from jax.experimental.shard_map import shard_map
    from jax.sharding import Mesh, PartitionSpec, NamedSharding
    from concourse import bass2jax, mybir
    from concourse.bass2jax import (_bass_exec_p, install_neuronx_cc_hook,
                                    partition_id_tensor)

    install_neuronx_cc_hook()
    partition_name = (nc.partition_id_tensor.name
                      if nc.partition_id_tensor else None)
    in_names, out_names, out_avals = [], [], []
    for alloc in nc.m.functions[0].allocations:
        if not isinstance(alloc, mybir.MemoryLocationSet):
            continue
        name = alloc.memorylocations[0].name
        if alloc.kind == "ExternalInput":
            if name != partition_name:
                in_names.append(name)
        elif alloc.kind == "ExternalOutput":
            out_names.append(name)
            out_avals.append(jax.core.ShapedArray(
                tuple(alloc.tensor_shape), mybir.dt.np(alloc.dtype)))
    n_params = len(in_names)
    all_in = in_names + out_names
    if partition_name is not None:
        all_in = all_in + [partition_name]
    donate = tuple(range(n_params, n_params + len(out_names)))

    def _body(*args):
        operands = list(args)
        if partition_name is not None:
            operands.append(partition_id_tensor())
        outs = _bass_exec_p.bind(
            *operands,
            out_avals=tuple(out_avals),
            in_names=tuple(all_in),
            out_names=tuple(out_names),
            lowering_input_output_aliases=(),
            sim_require_finite=True,
            sim_require_nnan=True,
            nc=nc,
        )
        return tuple(outs)

    devices = jax.devices()[:n_cores]
    mesh = Mesh(np.asarray(devices), ("core",))
    n_all = n_params + len(out_names)
    sm = shard_map(_body, mesh=mesh,
                   in_specs=(PartitionSpec("core"),) * n_all,
                   out_specs=(PartitionSpec("core"),) * len(out_names),
                   check_rep=False)
    run_fn = jax.jit(sm, donate_argnums=donate, keep_unused=True)
    bench_fn = jax.jit(sm, keep_unused=True)
    sharding = NamedSharding(mesh, PartitionSpec("core"))
    _RUNNER = dict(run=run_fn, bench=bench_fn, in_names=in_names,
                   out_names=out_names, out_avals=out_avals, mesh=mesh,
                   sharding=sharding, n_cores=n_cores)
    return _RUNNER


def _concat_inputs(runner, in_maps):
    n_cores = runner["n_cores"]
    concat_in = [
        np.concatenate([np.asarray(in_maps[c][name]) for c in range(n_cores)], axis=0)
        for name in runner["in_names"]
    ]
    concat_zeros = [
        np.zeros((n_cores * a.shape[0], *a.shape[1:]), a.dtype)
        for a in runner["out_avals"]
    ]
    return concat_in, concat_zeros


def _run(nc, in_maps, n_cores):
    runner = _get_runner(nc, n_cores)
    concat_in, concat_zeros = _concat_inputs(runner, in_maps)
    out_arrs = runner["run"](*concat_in, *concat_zeros)
    out_avals = runner["out_avals"]
    return [
        {name: np.asarray(out_arrs[i]).reshape(n_cores, *out_avals[i].shape)[c]
         for i, name in enumerate(runner["out_names"])}
        for c in range(n_cores)
    ]


def bench(in_maps=None, n_iters=20):
    """Time the compiled NEFF with device-resident inputs. Returns list of
    per-call seconds (min ≈ HW exec + dispatch overhead)."""
    import time
    import jax
    runner = _RUNNER
    assert runner is not None, "call kernel() once first"
    global _BENCH_ARGS
    if in_maps is not None:
        concat_in, concat_zeros = _concat_inputs(runner, in_maps)
        _BENCH_ARGS = [jax.device_put(a, runner["sharding"])
                       for a in concat_in + concat_zeros]
        jax.block_until_ready(_BENCH_ARGS)
    args = _BENCH_ARGS
    fn = runner["bench"]
    jax.block_until_ready(fn(*args))  # warm compile
    times = []
    for _ in range(n_iters):
        t0 = time.time()
        jax.block_until_ready(fn(*args))
        times.append(time.time() - t0)
    return times
